# revision 18
# baseline (speedup 1.0000x reference)
"""Trainium2 Bass kernel for nn_AttentionBlock (B=8, S=2048, DIM_VAL=DIM_ATTN=512).

Sharding: pure data parallelism — batch element b runs on NeuronCore b (B=8 = n_cores).

Host-side input marshaling (no sequence-dependent FLOPs on host):
  - x is sharded per-core and pre-transposed to xT [D, S]
  - Wq/Wk are folded into M = Wq^T @ Wk  [D, D]  (weight-only preprocessing), since
    scores = (x Wq^T)(x Wk^T)^T = x · M · x^T
  - Wv is pre-transposed to WvT [D, D]

Per-core dataflow (single batch element, S=2048, D=A=512):
  phase 1: YT[d', s] = M.T-chunks @ xT    ( Y = x @ M, transposed layout )
           V [s, v]  = xT-chunk.T @ WvT   ( s on partitions — ready for attn@V )
  phase 2: per q-chunk of 512 query positions:
           ET[s_k, s_q] = exp((xT.T-chunks @ YT_chunk) * 1/sqrt(A))  # scores transposed
                                                                     # so ET feeds attn@V
           r[s_q]: DVE pre-accumulates the 16 ET k-tiles, one ones-matmul
                    contracts the last 128 partitions; bounce r through DRAM
                    to realign partition-wise with U's rows
           U[s_q, v]    = ET.T @ V ;  out = U * (1/r)  fused into PSUM->SBUF copy
All matmul operands are fp16 (PSUM accumulation stays fp32); end-to-end error vs
the fp32 reference is ~4e-4 (absmax-normalized). HW exec ~165-210us on 8 cores
(run-to-run spread comes from the PE P0 power-state clock, 2.0 vs 2.4 GHz).
"""

import os
import sys
from contextlib import ExitStack

import numpy as np

for _p in ("/root/.axon_site/_ro/trn_rl_repo", "/opt/trn_rl_repo"):
    if os.path.isdir(_p) and _p not in sys.path:
        sys.path.append(_p)

import concourse.bacc as bacc
import concourse.bass as bass
import concourse.mybir as mybir
import concourse.tile as tile
from concourse.bass_utils import run_bass_kernel_spmd

B, S, D, A = 8, 2048, 512, 512
P = 128
N_CORES = 8
FP32 = mybir.dt.float32

# fp16: full-rate matmul (1 cyc/row) with FWL weight loads that hide fully under
# the matmul stream — unlike fp32r whose fused 4-byte weight reload (~216ns) paces
# every MM to ~272ns. 10-bit mantissa keeps end-to-end error ~4e-4 (measured).
MM_DT = mybir.dt.float16

SD = S // P        # 16 s-tiles of 128
DC = D // P        # 4 d-chunks of 128
NQ = 4             # q-chunks of 512
QW = S // NQ       # 512 — q-chunk width
KC = SD            # 16 k-chunks of 128


def build_body(ctx: ExitStack, tc: tile.TileContext, xt_d, m_d, wvt_d, out_d):
    nc = tc.nc

    const = ctx.enter_context(tc.tile_pool(name="const", bufs=1))
    ones = const.tile([P, 1], MM_DT)
    nc.vector.memset(ones[:], 1.0)

    # Persistent per-core tensors (live across both phases)
    persist = ctx.enter_context(tc.tile_pool(name="persist", bufs=1))
    xT = persist.tile([P, DC, S], MM_DT)     # [d%128, d//128, s] — also scores lhsT
    YT = persist.tile([P, DC, S], MM_DT)     # [d'%128, d'//128, s]
    V = persist.tile([P, SD, D], MM_DT)      # [s%128, s//128, v]

    dram = ctx.enter_context(tc.tile_pool(name="dram", bufs=2, space="DRAM"))

    # One PSUM pool layout for the whole kernel: a pool boundary between the
    # projection and attention phases serializes the first scores matmul behind
    # the last projection copy-out (pool alloc depends on pool release).
    ps_sc = ctx.enter_context(tc.tile_pool(name="ps_sc", bufs=5, space="PSUM"))
    ps_u = ctx.enter_context(tc.tile_pool(name="ps_u", bufs=3, space="PSUM"))

    # ---------------- phase 1: projections ----------------
    with (
        tc.tile_pool(name="p1_w", bufs=1) as wtp,
    ):
        # Dummy matmuls spanning the input-DMA window: keeps the PE busy through
        # the HAM activity window so the real matmul stream starts at 2.4 GHz.
        dummy = const.tile([P, QW], MM_DT, tag="dummy")
        nc.vector.memset(dummy[:], 0.0)
        # preload the exp table-set now (one-time ~1.5us ACT_TABLE_LOAD) instead
        # of mid-stream at the first real exp
        dume = const.tile([P, 1], FP32, tag="dume")
        nc.scalar.activation(dume[:], dummy[:, 0:1],
                             mybir.ActivationFunctionType.Exp, scale=1.0)
        pw = ps_sc.tile([P, QW], FP32, tag="sc")
        for _ in range(22):
            nc.tensor.matmul(pw[:], dummy[:, 0:P], dummy[:], start=True, stop=True)
        # weights first (small), then x by s-chunk so compute starts early
        mt = wtp.tile([P, DC, D], MM_DT, tag="m")
        m_src = m_d.rearrange("(c p) o -> p c o", p=P)
        xt_src = xt_d.rearrange("(c p) s -> p c s", p=P)
        # critical prefix first: the opening projection group reads only
        # M[:, :, 0:128] and xT[:, :, 0:512]
        nc.sync.dma_start(mt[:, :, 0:P], m_src[:, :, 0:P])
        nc.sync.dma_start(xT[:, :, 0:QW], xt_src[:, :, 0:QW])
        for oi in range(1, DC):
            osl = slice(oi * P, (oi + 1) * P)
            nc.sync.dma_start(mt[:, :, osl], m_src[:, :, osl])
        wv = wtp.tile([P, DC, D], MM_DT, tag="wv")
        nc.sync.dma_start(wv[:], wvt_d.rearrange("(c p) o -> p c o", p=P))
        for sj in range(1, NQ):
            ssl = slice(sj * QW, (sj + 1) * QW)
            nc.sync.dma_start(xT[:, :, ssl], xt_src[:, :, ssl])

        # YT[d', s]: lhsT = M[d, d'-chunk], rhs = xT[d, s-chunk(512)]
        for sj in range(NQ):
            ssl = slice(sj * QW, (sj + 1) * QW)
            for oi in range(DC):
                pm = ps_sc.tile([P, QW], FP32, tag="sc")
                for dk in range(DC):
                    nc.tensor.matmul(
                        pm[:],
                        mt[:, dk, oi * P:(oi + 1) * P],
                        xT[:, dk, ssl],
                        start=(dk == 0),
                        stop=(dk == DC - 1),
                    )
                nc.vector.tensor_copy(YT[:, oi, ssl], pm[:])

        # V[s, v]: lhsT = xT[d, s-chunk(128)], rhs = WvT[d, v(512)]
        for si in range(SD):
            pm = ps_sc.tile([P, D], FP32, tag="sc")
            for dk in range(DC):
                nc.tensor.matmul(
                    pm[:],
                    xT[:, dk, si * P:(si + 1) * P],
                    wv[:, dk, :],
                    start=(dk == 0),
                    stop=(dk == DC - 1),
                )
            nc.vector.tensor_copy(V[:, si, :], pm[:])

    # ---------------- phase 2: attention, per q-chunk of 512 ----------------
    inv_sqrt_a = 1.0 / float(np.sqrt(A))
    with (
        tc.tile_pool(name="et", bufs=2) as etp,
        tc.tile_pool(name="p2_small", bufs=2) as p2s,
        tc.tile_pool(name="p2_out", bufs=3) as p2o,
    ):
        for qc in range(NQ):
            qsl = slice(qc * QW, (qc + 1) * QW)

            # scoresT[s_k, s_q] = xT.T @ YT, then exp -> ET
            et = etp.tile([P, KC, QW], MM_DT, tag="et")
            for ki in range(KC):
                pm = ps_sc.tile([P, QW], FP32, tag="sc")
                for dk in range(DC):
                    nc.tensor.matmul(
                        pm[:],
                        xT[:, dk, ki * P:(ki + 1) * P],
                        YT[:, dk, qsl],
                        start=(dk == 0),
                        stop=(dk == DC - 1),
                    )
                nc.scalar.activation(
                    et[:, ki, :], pm[:],
                    mybir.ActivationFunctionType.Exp,
                    scale=inv_sqrt_a,
                )

            # rowsums r[s_q] = sum_k ET[k, s_q]:
            # DVE pre-accumulates the 16 k-chunk tiles (fp32 accumulator, last
            # add writes the fp16 operand tile), then one ones-matmul contracts
            # the remaining 128 partitions. The ones-matmul is emitted AFTER
            # U(c=0) so the DVE chain never head-of-line-blocks the PE queue.
            acc = p2s.tile([P, QW], FP32, tag="acc")
            nc.vector.tensor_copy(acc[:], et[:, 0, :])
            for ki in range(1, KC - 1):
                nc.vector.tensor_add(out=acc[:], in0=acc[:], in1=et[:, ki, :])
            acc16 = p2s.tile([P, QW], MM_DT, tag="acc16")
            nc.vector.tensor_add(out=acc16[:], in0=acc[:], in1=et[:, KC - 1, :])

            def u_matmuls(c):
                pu = ps_u.tile([P, D], FP32, tag="u")
                for ki in range(KC):
                    nc.tensor.matmul(
                        pu[:],
                        et[:, ki, c * P:(c + 1) * P],
                        V[:, ki, :],
                        start=(ki == 0),
                        stop=(ki == KC - 1),
                    )
                return pu

            pu0 = u_matmuls(0)

            pr = ps_sc.tile([P, QW], FP32, tag="sc")  # rowsum borrows a scores slot
            nc.tensor.matmul(pr[0:1, :], ones[:], acc16[:], start=True, stop=True)
            r_sb = p2s.tile([1, QW], FP32, tag="r_sb")
            nc.vector.tensor_copy(r_sb[:], pr[0:1, :])
            # bounce through DRAM to realign: r[128 rows of this q-chunk, 4 subchunks]
            r_dram = dram.tile([1, QW], FP32, tag="r_dram")
            nc.sync.dma_start(r_dram[:], r_sb[:])
            r_part = p2s.tile([P, NQ], FP32, tag="r_part")
            nc.sync.dma_start(
                r_part[:], r_dram[:].rearrange("one (c p) -> (one p) c", p=P)
            )
            rinv = p2s.tile([P, NQ], FP32, tag="rinv")
            nc.vector.reciprocal(rinv[:], r_part[:])

            def u_out(c, pu):
                o_sb = p2o.tile([P, D], FP32, tag="o_sb")
                nc.vector.tensor_scalar_mul(o_sb[:], pu[:], rinv[:, c:c + 1])
                row0 = qc * QW + c * P
                nc.sync.dma_start(out_d[row0:row0 + P, :], o_sb[:])

            u_out(0, pu0)
            for c in range(1, NQ):
                u_out(c, u_matmuls(c))


def build_program() -> bass.Bass:
    nc = bacc.Bacc("TRN2", target_bir_lowering=False, debug=False,
                   num_devices=N_CORES)
    # host pre-rounds inputs to fp16 — DMA moves fp16 bits directly
    xt_d = nc.dram_tensor("xT", [D, S], MM_DT, kind="ExternalInput").ap()
    m_d = nc.dram_tensor("M", [D, D], MM_DT, kind="ExternalInput").ap()
    wvt_d = nc.dram_tensor("WvT", [D, D], MM_DT, kind="ExternalInput").ap()
    out_d = nc.dram_tensor("out", [S, D], FP32, kind="ExternalOutput").ap()
    with tile.TileContext(nc) as tc:
        with ExitStack() as ctx:
            build_body(ctx, tc, xt_d, m_d, wvt_d, out_d)
    nc.compile()
    return nc


_prog_cache = {}


def _get_program() -> bass.Bass:
    if "nc" not in _prog_cache:
        _prog_cache["nc"] = build_program()
    return _prog_cache["nc"]


def make_in_maps(x, Wq, Wk, Wv):
    x = np.asarray(x, dtype=np.float32)
    Wq = np.asarray(Wq, dtype=np.float32)
    Wk = np.asarray(Wk, dtype=np.float32)
    # weight-only folding: scores = x M x^T with M = Wq^T Wk (fp32 on host)
    M = np.ascontiguousarray((Wq.T @ Wk).astype(np.float16))
    WvT = np.ascontiguousarray(np.asarray(Wv, dtype=np.float32).T.astype(np.float16))
    return [
        {"xT": np.ascontiguousarray(x[i].T.astype(np.float16)), "M": M, "WvT": WvT}
        for i in range(N_CORES)
    ]


def run_spmd(x, Wq, Wk, Wv, **kw):
    nc = _get_program()
    return run_bass_kernel_spmd(nc, make_in_maps(x, Wq, Wk, Wv),
                                list(range(N_CORES)), **kw)


def kernel(x, Wq, Wk, Wv):
    res = run_spmd(x, Wq, Wk, Wv)
    return np.stack([res.results[i]["out"] for i in range(N_CORES)], axis=0)


# revision 19
# speedup vs baseline: 1.1718x; 1.1718x over previous
"""Trainium2 Bass kernel for nn_AttentionBlock (B=8, S=2048, DIM_VAL=DIM_ATTN=512).

Sharding: pure data parallelism — batch element b runs on NeuronCore b (B=8 = n_cores).

Host-side input marshaling (no sequence-dependent FLOPs on host):
  - x is sharded per-core and pre-transposed to xT [D, S]
  - Wq/Wk are folded into M = Wq^T @ Wk  [D, D]  (weight-only preprocessing), since
    scores = (x Wq^T)(x Wk^T)^T = x · M · x^T
  - Wv is pre-transposed to WvT [D, D]

Per-core dataflow (single batch element, S=2048, D=A=512):
  phase 1: YT[d', s] = M.T-chunks @ xT    ( Y = x @ M, transposed layout )
           V [s, v]  = xT-chunk.T @ WvT   ( s on partitions — ready for attn@V )
  phase 2: per q-chunk of 512 query positions:
           ET[s_k, s_q] = exp((xT.T-chunks @ YT_chunk) * 1/sqrt(A))  # scores transposed
                                                                     # so ET feeds attn@V
           r[s_q]: DVE pre-accumulates the 16 ET k-tiles, one ones-matmul
                    contracts the last 128 partitions; bounce r through DRAM
                    to realign partition-wise with U's rows
           U[s_q, v]    = ET.T @ V ;  out = U * (1/r)  fused into PSUM->SBUF copy
All matmul operands are fp16 (PSUM accumulation stays fp32); end-to-end error vs
the fp32 reference is ~4e-4 (absmax-normalized). HW exec ~165-210us on 8 cores
(run-to-run spread comes from the PE P0 power-state clock, 2.0 vs 2.4 GHz).
"""

import os
import sys
from contextlib import ExitStack

import numpy as np

for _p in ("/root/.axon_site/_ro/trn_rl_repo", "/opt/trn_rl_repo"):
    if os.path.isdir(_p) and _p not in sys.path:
        sys.path.append(_p)

import concourse.bacc as bacc
import concourse.bass as bass
import concourse.mybir as mybir
import concourse.tile as tile
from concourse.bass_utils import run_bass_kernel_spmd
from concourse.tile_rust import add_dep_helper

B, S, D, A = 8, 2048, 512, 512
P = 128
N_CORES = 8
FP32 = mybir.dt.float32

# fp16: full-rate matmul (1 cyc/row) with FWL weight loads that hide fully under
# the matmul stream — unlike fp32r whose fused 4-byte weight reload (~216ns) paces
# every MM to ~272ns. 10-bit mantissa keeps end-to-end error ~4e-4 (measured).
MM_DT = mybir.dt.float16

SD = S // P        # 16 s-tiles of 128
DC = D // P        # 4 d-chunks of 128
NQ = 4             # q-chunks of 512
QW = S // NQ       # 512 — q-chunk width
KC = SD            # 16 k-chunks of 128


def build_body(ctx: ExitStack, tc: tile.TileContext, xt_d, m_d, wvt_d, out_d):
    nc = tc.nc

    const = ctx.enter_context(tc.tile_pool(name="const", bufs=1))
    ones = const.tile([P, 1], MM_DT)
    nc.vector.memset(ones[:], 1.0)

    # Persistent per-core tensors (live across both phases)
    persist = ctx.enter_context(tc.tile_pool(name="persist", bufs=1))
    xT = persist.tile([P, DC, S], MM_DT)     # [d%128, d//128, s] — also scores lhsT
    YT = persist.tile([P, DC, S], MM_DT)     # [d'%128, d'//128, s]
    V = persist.tile([P, SD, D], MM_DT)      # [s%128, s//128, v]

    dram = ctx.enter_context(tc.tile_pool(name="dram", bufs=2, space="DRAM"))

    # One PSUM pool layout for the whole kernel: a pool boundary between the
    # projection and attention phases serializes the first scores matmul behind
    # the last projection copy-out (pool alloc depends on pool release).
    ps_sc = ctx.enter_context(tc.tile_pool(name="ps_sc", bufs=5, space="PSUM"))
    ps_u = ctx.enter_context(tc.tile_pool(name="ps_u", bufs=3, space="PSUM"))

    # ---------------- phase 1: projections ----------------
    with (
        tc.tile_pool(name="p1_w", bufs=1) as wtp,
    ):
        # Dummy matmuls spanning the input-DMA window: keeps the PE busy through
        # the HAM activity window so the real matmul stream starts at 2.4 GHz.
        dummy = const.tile([P, QW], MM_DT, tag="dummy")
        nc.vector.memset(dummy[:], 0.0)
        # preload the exp table-set now (one-time ~1.5us ACT_TABLE_LOAD) instead
        # of mid-stream at the first real exp
        dume = const.tile([P, 1], FP32, tag="dume")
        nc.scalar.activation(dume[:], dummy[:, 0:1],
                             mybir.ActivationFunctionType.Exp, scale=1.0)
        pw = ps_sc.tile([P, QW], FP32, tag="sc")
        for _ in range(20):
            nc.tensor.matmul(pw[:], dummy[:, 0:P], dummy[:], start=True, stop=True)
        # weights first (small), then x by s-chunk so compute starts early
        mt = wtp.tile([P, DC, D], MM_DT, tag="m")
        m_src = m_d.rearrange("(c p) o -> p c o", p=P)
        xt_src = xt_d.rearrange("(c p) s -> p c s", p=P)
        # critical prefix first: the opening projection group reads only
        # M[:, :, 0:128] and xT[:, :, 0:512]. All other input DMAs depend on the
        # critical pair so their packets don't steal its DMA bandwidth.
        nc.sync.dma_start(mt[:, :, 0:P], m_src[:, :, 0:P])
        crit = nc.sync.dma_start(xT[:, :, 0:QW], xt_src[:, :, 0:QW])
        bulk = []
        for oi in range(1, DC):
            osl = slice(oi * P, (oi + 1) * P)
            bulk.append(nc.sync.dma_start(mt[:, :, osl], m_src[:, :, osl]))
        for sj in range(1, NQ):
            ssl = slice(sj * QW, (sj + 1) * QW)
            bulk.append(nc.sync.dma_start(xT[:, :, ssl], xt_src[:, :, ssl]))
        wv = wtp.tile([P, DC, D], MM_DT, tag="wv")
        bulk.append(nc.sync.dma_start(wv[:], wvt_d.rearrange("(c p) o -> p c o", p=P)))
        for b in bulk:
            add_dep_helper(b.ins, crit.ins,
                           reason="bulk input DMA deferred behind critical prefix")

        # YT[d', s]: lhsT = M[d, d'-chunk], rhs = xT[d, s-chunk(512)]
        for sj in range(NQ):
            ssl = slice(sj * QW, (sj + 1) * QW)
            for oi in range(DC):
                pm = ps_sc.tile([P, QW], FP32, tag="sc")
                for dk in range(DC):
                    nc.tensor.matmul(
                        pm[:],
                        mt[:, dk, oi * P:(oi + 1) * P],
                        xT[:, dk, ssl],
                        start=(dk == 0),
                        stop=(dk == DC - 1),
                    )
                nc.vector.tensor_copy(YT[:, oi, ssl], pm[:])

        # V[s, v]: lhsT = xT[d, s-chunk(128)], rhs = WvT[d, v(512)]
        for si in range(SD):
            pm = ps_sc.tile([P, D], FP32, tag="sc")
            for dk in range(DC):
                nc.tensor.matmul(
                    pm[:],
                    xT[:, dk, si * P:(si + 1) * P],
                    wv[:, dk, :],
                    start=(dk == 0),
                    stop=(dk == DC - 1),
                )
            nc.vector.tensor_copy(V[:, si, :], pm[:])

    # ---------------- phase 2: attention, per q-chunk of 512 ----------------
    inv_sqrt_a = 1.0 / float(np.sqrt(A))
    with (
        tc.tile_pool(name="et", bufs=2) as etp,
        tc.tile_pool(name="p2_small", bufs=2) as p2s,
        tc.tile_pool(name="p2_out", bufs=3) as p2o,
    ):
        for qc in range(NQ):
            qsl = slice(qc * QW, (qc + 1) * QW)

            # scoresT[s_k, s_q] = xT.T @ YT, then exp -> ET
            et = etp.tile([P, KC, QW], MM_DT, tag="et")
            for ki in range(KC):
                pm = ps_sc.tile([P, QW], FP32, tag="sc")
                for dk in range(DC):
                    nc.tensor.matmul(
                        pm[:],
                        xT[:, dk, ki * P:(ki + 1) * P],
                        YT[:, dk, qsl],
                        start=(dk == 0),
                        stop=(dk == DC - 1),
                    )
                nc.scalar.activation(
                    et[:, ki, :], pm[:],
                    mybir.ActivationFunctionType.Exp,
                    scale=inv_sqrt_a,
                )

            # rowsums r[s_q] = sum_k ET[k, s_q]:
            # DVE pre-accumulates the 16 k-chunk tiles (fp32 accumulator, last
            # add writes the fp16 operand tile), then one ones-matmul contracts
            # the remaining 128 partitions. The ones-matmul is emitted AFTER
            # U(c=0) so the DVE chain never head-of-line-blocks the PE queue.
            acc = p2s.tile([P, QW], FP32, tag="acc")
            nc.vector.tensor_copy(acc[:], et[:, 0, :])
            for ki in range(1, KC - 1):
                nc.vector.tensor_add(out=acc[:], in0=acc[:], in1=et[:, ki, :])
            acc16 = p2s.tile([P, QW], MM_DT, tag="acc16")
            nc.vector.tensor_add(out=acc16[:], in0=acc[:], in1=et[:, KC - 1, :])

            def u_matmuls(c):
                pu = ps_u.tile([P, D], FP32, tag="u")
                for ki in range(KC):
                    nc.tensor.matmul(
                        pu[:],
                        et[:, ki, c * P:(c + 1) * P],
                        V[:, ki, :],
                        start=(ki == 0),
                        stop=(ki == KC - 1),
                    )
                return pu

            pu0 = u_matmuls(0)

            pr = ps_sc.tile([P, QW], FP32, tag="sc")  # rowsum borrows a scores slot
            nc.tensor.matmul(pr[0:1, :], ones[:], acc16[:], start=True, stop=True)
            r_sb = p2s.tile([1, QW], FP32, tag="r_sb")
            nc.vector.tensor_copy(r_sb[:], pr[0:1, :])
            # bounce through DRAM to realign: r[128 rows of this q-chunk, 4 subchunks]
            r_dram = dram.tile([1, QW], FP32, tag="r_dram")
            nc.sync.dma_start(r_dram[:], r_sb[:])
            r_part = p2s.tile([P, NQ], FP32, tag="r_part")
            nc.sync.dma_start(
                r_part[:], r_dram[:].rearrange("one (c p) -> (one p) c", p=P)
            )
            rinv = p2s.tile([P, NQ], FP32, tag="rinv")
            nc.vector.reciprocal(rinv[:], r_part[:])

            def u_out(c, pu):
                o_sb = p2o.tile([P, D], FP32, tag="o_sb")
                nc.vector.tensor_scalar_mul(o_sb[:], pu[:], rinv[:, c:c + 1])
                row0 = qc * QW + c * P
                nc.sync.dma_start(out_d[row0:row0 + P, :], o_sb[:])

            u_out(0, pu0)
            for c in range(1, NQ):
                u_out(c, u_matmuls(c))


def build_program() -> bass.Bass:
    nc = bacc.Bacc("TRN2", target_bir_lowering=False, debug=False,
                   num_devices=N_CORES)
    # host pre-rounds inputs to fp16 — DMA moves fp16 bits directly
    xt_d = nc.dram_tensor("xT", [D, S], MM_DT, kind="ExternalInput").ap()
    m_d = nc.dram_tensor("M", [D, D], MM_DT, kind="ExternalInput").ap()
    wvt_d = nc.dram_tensor("WvT", [D, D], MM_DT, kind="ExternalInput").ap()
    out_d = nc.dram_tensor("out", [S, D], FP32, kind="ExternalOutput").ap()
    with tile.TileContext(nc) as tc:
        with ExitStack() as ctx:
            build_body(ctx, tc, xt_d, m_d, wvt_d, out_d)
    nc.compile()
    return nc


_prog_cache = {}


def _get_program() -> bass.Bass:
    if "nc" not in _prog_cache:
        _prog_cache["nc"] = build_program()
    return _prog_cache["nc"]


def make_in_maps(x, Wq, Wk, Wv):
    x = np.asarray(x, dtype=np.float32)
    Wq = np.asarray(Wq, dtype=np.float32)
    Wk = np.asarray(Wk, dtype=np.float32)
    # weight-only folding: scores = x M x^T with M = Wq^T Wk (fp32 on host)
    M = np.ascontiguousarray((Wq.T @ Wk).astype(np.float16))
    WvT = np.ascontiguousarray(np.asarray(Wv, dtype=np.float32).T.astype(np.float16))
    return [
        {"xT": np.ascontiguousarray(x[i].T.astype(np.float16)), "M": M, "WvT": WvT}
        for i in range(N_CORES)
    ]


def run_spmd(x, Wq, Wk, Wv, **kw):
    nc = _get_program()
    return run_bass_kernel_spmd(nc, make_in_maps(x, Wq, Wk, Wv),
                                list(range(N_CORES)), **kw)


def kernel(x, Wq, Wk, Wv):
    res = run_spmd(x, Wq, Wk, Wv)
    return np.stack([res.results[i]["out"] for i in range(N_CORES)], axis=0)


# revision 20
# speedup vs baseline: 1.2011x; 1.0250x over previous
"""Trainium2 Bass kernel for nn_AttentionBlock (B=8, S=2048, DIM_VAL=DIM_ATTN=512).

Sharding: pure data parallelism — batch element b runs on NeuronCore b (B=8 = n_cores).

Host-side input marshaling (no sequence-dependent FLOPs on host):
  - x is sharded per-core and pre-transposed to xT [D, S]
  - Wq/Wk are folded into M = Wq^T @ Wk  [D, D]  (weight-only preprocessing), since
    scores = (x Wq^T)(x Wk^T)^T = x · M · x^T
  - Wv is pre-transposed to WvT [D, D]

Per-core dataflow (single batch element, S=2048, D=A=512):
  phase 1: YT[d', s] = M.T-chunks @ xT    ( Y = x @ M, transposed layout )
           V [s, v]  = xT-chunk.T @ WvT   ( s on partitions — ready for attn@V )
  phase 2: per q-chunk of 512 query positions:
           ET[s_k, s_q] = exp((xT.T-chunks @ YT_chunk) * 1/sqrt(A))  # scores transposed
                                                                     # so ET feeds attn@V
           r[s_q]: DVE pre-accumulates the 16 ET k-tiles, one ones-matmul
                    contracts the last 128 partitions; bounce r through DRAM
                    to realign partition-wise with U's rows
           U[s_q, v]    = ET.T @ V ;  out = U * (1/r)  fused into PSUM->SBUF copy
All matmul operands are fp16 (PSUM accumulation stays fp32); end-to-end error vs
the fp32 reference is ~4e-4 (absmax-normalized). HW exec ~165-210us on 8 cores
(run-to-run spread comes from the PE P0 power-state clock, 2.0 vs 2.4 GHz).
"""

import os
import sys
from contextlib import ExitStack

import numpy as np

for _p in ("/root/.axon_site/_ro/trn_rl_repo", "/opt/trn_rl_repo"):
    if os.path.isdir(_p) and _p not in sys.path:
        sys.path.append(_p)

import concourse.bacc as bacc
import concourse.bass as bass
import concourse.mybir as mybir
import concourse.tile as tile
from concourse.bass_utils import run_bass_kernel_spmd

B, S, D, A = 8, 2048, 512, 512
P = 128
N_CORES = 8
FP32 = mybir.dt.float32

# fp16: full-rate matmul (1 cyc/row) with FWL weight loads that hide fully under
# the matmul stream — unlike fp32r whose fused 4-byte weight reload (~216ns) paces
# every MM to ~272ns. 10-bit mantissa keeps end-to-end error ~4e-4 (measured).
MM_DT = mybir.dt.float16

SD = S // P        # 16 s-tiles of 128
DC = D // P        # 4 d-chunks of 128
NQ = 4             # q-chunks of 512
QW = S // NQ       # 512 — q-chunk width
KC = SD            # 16 k-chunks of 128


def build_body(ctx: ExitStack, tc: tile.TileContext, xt_d, m_d, wvt_d, out_d):
    nc = tc.nc

    const = ctx.enter_context(tc.tile_pool(name="const", bufs=1))
    ones = const.tile([P, 1], MM_DT)
    nc.vector.memset(ones[:], 1.0)

    # Persistent per-core tensors (live across both phases)
    persist = ctx.enter_context(tc.tile_pool(name="persist", bufs=1))
    xT = persist.tile([P, DC, S], MM_DT)     # [d%128, d//128, s] — also scores lhsT
    YT = persist.tile([P, DC, S], MM_DT)     # [d'%128, d'//128, s]
    V = persist.tile([P, SD, D], MM_DT)      # [s%128, s//128, v]

    dram = ctx.enter_context(tc.tile_pool(name="dram", bufs=2, space="DRAM"))

    # One PSUM pool layout for the whole kernel: a pool boundary between the
    # projection and attention phases serializes the first scores matmul behind
    # the last projection copy-out (pool alloc depends on pool release).
    ps_sc = ctx.enter_context(tc.tile_pool(name="ps_sc", bufs=5, space="PSUM"))
    ps_u = ctx.enter_context(tc.tile_pool(name="ps_u", bufs=3, space="PSUM"))

    # ---------------- phase 1: projections ----------------
    with (
        tc.tile_pool(name="p1_w", bufs=1) as wtp,
    ):
        # Dummy matmuls spanning the input-DMA window: keeps the PE busy through
        # the HAM activity window so the real matmul stream starts at 2.4 GHz.
        dummy = const.tile([P, QW], MM_DT, tag="dummy")
        nc.vector.memset(dummy[:], 0.0)
        # preload the exp table-set now (one-time ~1.5us ACT_TABLE_LOAD) instead
        # of mid-stream at the first real exp
        dume = const.tile([P, 1], FP32, tag="dume")
        nc.scalar.activation(dume[:], dummy[:, 0:1],
                             mybir.ActivationFunctionType.Exp, scale=1.0)
        pw = ps_sc.tile([P, QW], FP32, tag="sc")
        for _ in range(22):
            nc.tensor.matmul(pw[:], dummy[:, 0:P], dummy[:], start=True, stop=True)
        # weights first (small), then x by s-chunk so compute starts early
        mt = wtp.tile([P, DC, D], MM_DT, tag="m")
        m_src = m_d.rearrange("(c p) o -> p c o", p=P)
        xt_src = xt_d.rearrange("(c p) s -> p c s", p=P)
        # critical prefix first: the opening projection group reads only
        # M[:, :, 0:128] and xT[:, :, 0:512]
        nc.sync.dma_start(mt[:, :, 0:P], m_src[:, :, 0:P])
        nc.sync.dma_start(xT[:, :, 0:QW], xt_src[:, :, 0:QW])
        for oi in range(1, DC):
            osl = slice(oi * P, (oi + 1) * P)
            nc.sync.dma_start(mt[:, :, osl], m_src[:, :, osl])
        wv = wtp.tile([P, DC, D], MM_DT, tag="wv")
        nc.sync.dma_start(wv[:], wvt_d.rearrange("(c p) o -> p c o", p=P))
        for sj in range(1, NQ):
            ssl = slice(sj * QW, (sj + 1) * QW)
            nc.sync.dma_start(xT[:, :, ssl], xt_src[:, :, ssl])

        # YT[d', s]: lhsT = M[d, d'-chunk], rhs = xT[d, s-chunk(512)]
        for sj in range(NQ):
            ssl = slice(sj * QW, (sj + 1) * QW)
            for oi in range(DC):
                pm = ps_sc.tile([P, QW], FP32, tag="sc")
                for dk in range(DC):
                    nc.tensor.matmul(
                        pm[:],
                        mt[:, dk, oi * P:(oi + 1) * P],
                        xT[:, dk, ssl],
                        start=(dk == 0),
                        stop=(dk == DC - 1),
                    )
                nc.vector.tensor_copy(YT[:, oi, ssl], pm[:])

        # V[s, v]: lhsT = xT[d, s-chunk(128)], rhs = WvT[d, v(512)]
        for si in range(SD):
            pm = ps_sc.tile([P, D], FP32, tag="sc")
            for dk in range(DC):
                nc.tensor.matmul(
                    pm[:],
                    xT[:, dk, si * P:(si + 1) * P],
                    wv[:, dk, :],
                    start=(dk == 0),
                    stop=(dk == DC - 1),
                )
            nc.vector.tensor_copy(V[:, si, :], pm[:])

    # ---------------- phase 2: attention, per q-chunk of 512 ----------------
    inv_sqrt_a = 1.0 / float(np.sqrt(A))
    with (
        tc.tile_pool(name="et", bufs=2) as etp,
        tc.tile_pool(name="p2_small", bufs=2) as p2s,
        tc.tile_pool(name="p2_out", bufs=3) as p2o,
    ):
        for qc in range(NQ):
            qsl = slice(qc * QW, (qc + 1) * QW)

            # scoresT[s_k, s_q] = xT.T @ YT, then exp -> ET
            et = etp.tile([P, KC, QW], MM_DT, tag="et")
            for ki in range(KC):
                pm = ps_sc.tile([P, QW], FP32, tag="sc")
                for dk in range(DC):
                    nc.tensor.matmul(
                        pm[:],
                        xT[:, dk, ki * P:(ki + 1) * P],
                        YT[:, dk, qsl],
                        start=(dk == 0),
                        stop=(dk == DC - 1),
                    )
                nc.scalar.activation(
                    et[:, ki, :], pm[:],
                    mybir.ActivationFunctionType.Exp,
                    scale=inv_sqrt_a,
                )

            # rowsums r[s_q] = sum_k ET[k, s_q]:
            # DVE pre-accumulates the 16 k-chunk tiles (fp32 accumulator, last
            # add writes the fp16 operand tile), then one ones-matmul contracts
            # the remaining 128 partitions. The ones-matmul is emitted AFTER
            # U(c=0) so the DVE chain never head-of-line-blocks the PE queue.
            acc = p2s.tile([P, QW], FP32, tag="acc")
            nc.vector.tensor_copy(acc[:], et[:, 0, :])
            for ki in range(1, KC - 1):
                nc.vector.tensor_add(out=acc[:], in0=acc[:], in1=et[:, ki, :])
            acc16 = p2s.tile([P, QW], MM_DT, tag="acc16")
            nc.vector.tensor_add(out=acc16[:], in0=acc[:], in1=et[:, KC - 1, :])

            def u_matmuls(c):
                pu = ps_u.tile([P, D], FP32, tag="u")
                for ki in range(KC):
                    nc.tensor.matmul(
                        pu[:],
                        et[:, ki, c * P:(c + 1) * P],
                        V[:, ki, :],
                        start=(ki == 0),
                        stop=(ki == KC - 1),
                    )
                return pu

            pu0 = u_matmuls(0)

            pr = ps_sc.tile([P, QW], FP32, tag="sc")  # rowsum borrows a scores slot
            nc.tensor.matmul(pr[0:1, :], ones[:], acc16[:], start=True, stop=True)
            r_sb = p2s.tile([1, QW], FP32, tag="r_sb")
            nc.vector.tensor_copy(r_sb[:], pr[0:1, :])
            # bounce through DRAM to realign: r[128 rows of this q-chunk, 4 subchunks]
            r_dram = dram.tile([1, QW], FP32, tag="r_dram")
            nc.sync.dma_start(r_dram[:], r_sb[:])
            r_part = p2s.tile([P, NQ], FP32, tag="r_part")
            nc.sync.dma_start(
                r_part[:], r_dram[:].rearrange("one (c p) -> (one p) c", p=P)
            )
            rinv = p2s.tile([P, NQ], FP32, tag="rinv")
            nc.vector.reciprocal(rinv[:], r_part[:])

            def u_out(c, pu):
                o_sb = p2o.tile([P, D], FP32, tag="o_sb")
                nc.vector.tensor_scalar_mul(o_sb[:], pu[:], rinv[:, c:c + 1])
                row0 = qc * QW + c * P
                nc.sync.dma_start(out_d[row0:row0 + P, :], o_sb[:])

            u_out(0, pu0)
            for c in range(1, NQ):
                u_out(c, u_matmuls(c))


def build_program() -> bass.Bass:
    nc = bacc.Bacc("TRN2", target_bir_lowering=False, debug=False,
                   num_devices=N_CORES)
    # host pre-rounds inputs to fp16 — DMA moves fp16 bits directly
    xt_d = nc.dram_tensor("xT", [D, S], MM_DT, kind="ExternalInput").ap()
    m_d = nc.dram_tensor("M", [D, D], MM_DT, kind="ExternalInput").ap()
    wvt_d = nc.dram_tensor("WvT", [D, D], MM_DT, kind="ExternalInput").ap()
    out_d = nc.dram_tensor("out", [S, D], FP32, kind="ExternalOutput").ap()
    with tile.TileContext(nc) as tc:
        with ExitStack() as ctx:
            build_body(ctx, tc, xt_d, m_d, wvt_d, out_d)
    nc.compile()
    return nc


_prog_cache = {}


def _get_program() -> bass.Bass:
    if "nc" not in _prog_cache:
        _prog_cache["nc"] = build_program()
    return _prog_cache["nc"]


def make_in_maps(x, Wq, Wk, Wv):
    x = np.asarray(x, dtype=np.float32)
    Wq = np.asarray(Wq, dtype=np.float32)
    Wk = np.asarray(Wk, dtype=np.float32)
    # weight-only folding: scores = x M x^T with M = Wq^T Wk (fp32 on host)
    M = np.ascontiguousarray((Wq.T @ Wk).astype(np.float16))
    WvT = np.ascontiguousarray(np.asarray(Wv, dtype=np.float32).T.astype(np.float16))
    return [
        {"xT": np.ascontiguousarray(x[i].T.astype(np.float16)), "M": M, "WvT": WvT}
        for i in range(N_CORES)
    ]


def run_spmd(x, Wq, Wk, Wv, **kw):
    nc = _get_program()
    return run_bass_kernel_spmd(nc, make_in_maps(x, Wq, Wk, Wv),
                                list(range(N_CORES)), **kw)


def kernel(x, Wq, Wk, Wv):
    res = run_spmd(x, Wq, Wk, Wv)
    return np.stack([res.results[i]["out"] for i in range(N_CORES)], axis=0)
